# revision 24
# baseline (speedup 1.0000x reference)
"""GQA multi-head attention (b=2, s=2048, d=2048, 32 Q heads / 8 KV heads,
head_dim=64, RoPE, causal) on 8 Trainium2 NeuronCores.

Sharding: tensor-parallel over heads x data-parallel over batch.
Core c = 4*bi + g handles batch bi and head-group g (8 Q heads, 2 KV heads).
Each core computes a partial [2048, 2048] output (its head block times the
matching wo rows); the host sums the 4 partials per batch.

v2 layout notes (vs the chunk-serial v1):
  - Software-pipelined emission: attention(c) is emitted before proj(c+1)
    and wo(c-1), so the Tile scheduler always has independent PE filler
    (projection / output-projection matmuls) to run while the Act engine
    exps score tiles.  This keeps the PE dense -> no HAM re-throttle.
  - Score tiles for the two 64-partition head halves (sub0 at rows 0-63,
    sub1 at 64-127) are emitted back-to-back; their implied tile_position
    row bases (0 / 64) let the PE run them concurrently (row tiling).
  - One exp per key tile covers both subs ([128,1024] PSUM -> bf16 probs).
  - Softmax denominators: 1/den = exp(-ln(den)) on the Act engine (the
    natural_log set holds exp AND ln -> one table load), replacing the
    3.3us/call DVE iterative reciprocal.
  - PSUM budget (8 banks): scores 2x[128,1024] (4) + pv out [65,1024] (2)
    + proj/wo shared [128,512] x2 (2).
  - DMAs are emitted in first-use order (w k-slab, xt k-slab interleaved).
"""

import sys

if "/opt/trn_rl_repo" not in sys.path:
    sys.path.insert(0, "/opt/trn_rl_repo")

import numpy as np

import concourse.bass as bass  # noqa: F401  (import keeps bass registered)
import concourse.tile as tile
from concourse import bacc, mybir
from concourse.bass_utils import run_bass_kernel_spmd

F32 = mybir.dt.float32
BF16 = mybir.dt.bfloat16

S = 2048
D = 2048
NH = 32
NKV = 8
DH = 64
ROPE_BASE = 10000.0
N_CORES = 8
QH_PER_CORE = 8   # local q heads
KVH_PER_CORE = 2  # local kv heads
DQ = QH_PER_CORE * DH   # 512, per-core q width
DKV = KVH_PER_CORE * DH  # 128, per-core kv width

# module-level knobs the test harness can poke
RUN_KWARGS: dict = {}
LAST_RESULTS = None

_COMPILED = None


def _build(loop_n=1, phases=3, abl=0):
    nc = bacc.Bacc("TRN2", target_bir_lowering=False, debug=False)

    xt_d = nc.dram_tensor("xt", [D, S], BF16, kind="ExternalInput").ap()
    wall_d = nc.dram_tensor("wall", [128, 16 * 768], BF16, kind="ExternalInput").ap()
    wo_d = nc.dram_tensor("wo", [128, 4 * 2048], BF16, kind="ExternalInput").ap()
    cos_d = nc.dram_tensor("cos", [128, S], BF16, kind="ExternalInput").ap()
    sin_d = nc.dram_tensor("sin", [128, S], BF16, kind="ExternalInput").ap()
    tri_d = nc.dram_tensor("tri", [128, 128], BF16, kind="ExternalInput").ap()
    out_d = nc.dram_tensor("out", [S, D], BF16, kind="ExternalOutput").ap()

    import contextlib

    ENGS = (
        mybir.EngineType.PE,
        mybir.EngineType.Activation,
        mybir.EngineType.DVE,
        mybir.EngineType.SP,
        mybir.EngineType.Pool,
    )
    with tile.TileContext(nc) as tc:
        with (
            tc.For_i(0, loop_n, 1, hint_engines=ENGS)
            if loop_n > 1
            else contextlib.nullcontext()
        ):
            _phases(nc, tc, xt_d, wall_d, wo_d, cos_d, sin_d, tri_d, out_d, phases, abl)

    nc.compile()
    return nc


def _phases(nc, tc, xt_d, wall_d, wo_d, cos_d, sin_d, tri_d, out_d, phases=3, abl=0):
    Exp = mybir.ActivationFunctionType.Exp

    with (
        tc.tile_pool(name="big", bufs=1) as big,
        tc.tile_pool(name="ropep", bufs=3) as ropep,
        tc.tile_pool(name="probsp", bufs=4) as probsp,
        tc.tile_pool(name="pvp", bufs=2) as pvp,
        tc.tile_pool(name="nrm", bufs=1) as nrm,
        tc.tile_pool(name="p3", bufs=2) as p3,
        tc.tile_pool(name="psP", bufs=2, space="PSUM") as psP,
        tc.tile_pool(name="psS", bufs=2, space="PSUM") as psS,
        tc.tile_pool(name="psO", bufs=1, space="PSUM") as psO,
    ):
        qrot = big.tile([128, 4 * S], BF16)   # 4 m-tiles of [2 heads x 64, S]
        krot = big.tile([128, S], BF16)       # [2 kv heads x 64, S]
        vt_sb = big.tile([128, S], BF16)      # v^T staging [vdim, seq]
        vaug = big.tile([128, 2 * 16 * 128], BF16)  # [keys, vdim|ones] slots
        attn = big.tile([128, 4 * S], BF16)   # normalized attention, qrot layout
        tri_sb = big.tile([128, 128], BF16)
        cos_sb = big.tile([128, S], BF16)
        sin_sb = big.tile([128, S], BF16)
        w_sb = big.tile([128, 16 * 768], BF16)
        xt_sb = big.tile([128, 16 * 2048], BF16)
        wo_sb = big.tile([128, 4 * 2048], BF16)

        # ---- DMAs in first-use order, batched to cut trigger count ----
        # w is m-major ([128, 6*2048], slice (m,k) at m*2048+k*128): the
        # first m-tile's matmuls only wait for their own 512KB slab.
        # xt is loaded with 3D-AP DMAs: dst (p, k, s) col k*2048+s, src row
        # k*128+p -- one trigger per seq-chunk instead of 16.
        xt_dst = xt_sb[:].rearrange("p (k s) -> p k s", k=16)
        xt_src = xt_d.rearrange("(k p) s -> p k s", p=128)
        nc.sync.dma_start(w_sb[:, 5 * 2048 : 6 * 2048], wall_d[:, 5 * 2048 : 6 * 2048])
        for h in range(2):
            nc.sync.dma_start(
                xt_dst[:, h * 8 : (h + 1) * 8, 0:512],
                xt_src[:, h * 8 : (h + 1) * 8, 0:512],
            )
        nc.sync.dma_start(w_sb[:, 0 : 2048], wall_d[:, 0 : 2048])
        nc.sync.dma_start(cos_sb[:], cos_d[:])
        nc.sync.dma_start(sin_sb[:], sin_d[:])
        nc.sync.dma_start(tri_sb[:], tri_d[:])
        for m in range(1, 5):
            nc.sync.dma_start(
                w_sb[:, m * 2048 : (m + 1) * 2048],
                wall_d[:, m * 2048 : (m + 1) * 2048],
            )
        for sc in range(1, 4):
            nc.sync.dma_start(
                xt_dst[:, :, sc * 512 : (sc + 1) * 512],
                xt_src[:, :, sc * 512 : (sc + 1) * 512],
            )
        nc.sync.dma_start(wo_sb[:], wo_d[:])

        nc.vector.memset(vaug[:], 1.0)  # ones columns for the PV denominators
        # zero the score psum slots once so diagonal-group exps never see
        # uninitialized PSUM (stale *scores* later are bounded and unused)
        zs = []
        for z in range(2):
            zt = psS.tile([128, 1024], F32, tag="scps", name=f"zz{z}")
            nc.vector.memset(zt[:], 0.0)
            zs.append(zt)

        def rope_evac(dst, raw, c):
            # dst = raw * cos + shift32(raw * sin); all bf16 SBUF (DVE 2x).
            cs = cos_sb[:, c * 512 : (c + 1) * 512]
            m1 = ropep.tile([128, 512], BF16, tag="m1")
            m2 = ropep.tile([128, 512], BF16, tag="m2")
            nc.vector.tensor_tensor(m1[:], raw[:], cs, op=mybir.AluOpType.mult)
            for q in range(4):
                a, b2 = q * 32, (q ^ 1) * 32
                nc.vector.tensor_tensor(
                    m2[b2 : b2 + 32, :],
                    raw[a : a + 32, :],
                    sin_sb[a : a + 32, c * 512 : (c + 1) * 512],
                    op=mybir.AluOpType.mult,
                )
            nc.vector.tensor_tensor(dst, m1[:], m2[:], op=mybir.AluOpType.add)

        def proj_chunk(c):
            # v (m=5) first: its XBAR transposes are slow (1.2us each) and
            # gate the next attention chunk's PV matmuls.
            for m in (5, 0, 1, 2, 3, 4):
                ps = psP.tile([128, 512], F32, tag="pj", name=f"pj{m}_{c}")
                for k in range(16):
                    nc.tensor.matmul(
                        ps[:],
                        lhsT=w_sb[:, m * 2048 + k * 128 : m * 2048 + (k + 1) * 128],
                        rhs=xt_sb[:, k * 2048 + c * 512 : k * 2048 + (c + 1) * 512],
                        start=(k == 0),
                        stop=(k == 15),
                    )
                if m == 5:
                    # high prio: the XBAR transposes are slow and gate the
                    # next attention chunk's PV matmuls
                    with tc.high_priority():
                        nc.vector.tensor_copy(vt_sb[:, c * 512 : (c + 1) * 512], ps[:])
                        for kv in range(2):
                            for i in range(4 * c, 4 * c + 4):
                                base = (kv * 16 + i) * 128
                                nc.sync.dma_start(
                                    vaug[:, base : base + 64],
                                    vt_sb[kv * 64 : (kv + 1) * 64, i * 128 : (i + 1) * 128],
                                    transpose=True,
                                )
                else:
                    raw = ropep.tile([128, 512], BF16, tag="raw", name=f"rw{m}_{c}")
                    nc.vector.tensor_copy(raw[:], ps[:])
                    if m < 4:
                        dst = qrot[:, m * S + c * 512 : m * S + (c + 1) * 512]
                    else:
                        dst = krot[:, c * 512 : (c + 1) * 512]
                    rope_evac(dst, raw, c)
        def attention_chunk(c):
            n_keys = 4 * c + 4
            for m in range(4):
                out_ps = psO.tile([65, 1024], F32, tag="outps", name=f"ops{m}_{c}")
                q0 = qrot[0:64, m * S + c * 512 : m * S + (c + 1) * 512]
                q1 = qrot[64:128, m * S + c * 512 : m * S + (c + 1) * 512]

                def pv(i, pr, off):
                    # accumulate both subs' PV into out_ps halves
                    nc.tensor.matmul(
                        out_ps[:, off:512],
                        lhsT=vaug[:, i * 128 : i * 128 + 65],
                        rhs=pr[:, off:512],
                        start=(i == 0),
                        stop=(i == n_keys - 1),
                        skip_group_check=True,
                    )
                    nc.tensor.matmul(
                        out_ps[:, 512 + off : 1024],
                        lhsT=vaug[:, (16 + i) * 128 : (16 + i) * 128 + 65],
                        rhs=pr[:, 512 + off : 1024],
                        start=(i == 0),
                        stop=(i == n_keys - 1),
                        skip_group_check=True,
                    )

                # full (sub-diagonal) key tiles
                for g in range(4 * c):
                    sc_ps = psS.tile([128, 1024], F32, tag="scps", name=f"sc{m}_{c}_{g}")
                    nc.tensor.matmul(
                        sc_ps[:, 0:512],
                        lhsT=krot[0:64, g * 128 : (g + 1) * 128],
                        rhs=q0, start=True, stop=True,
                    )
                    nc.tensor.matmul(
                        sc_ps[:, 512:1024],
                        lhsT=krot[64:128, g * 128 : (g + 1) * 128],
                        rhs=q1, start=True, stop=True,
                    )
                    pr = probsp.tile([128, 1024], BF16, tag="pr", name=f"pr{m}_{c}_{g}")
                    nc.scalar.activation(
                        pr[:], sc_ps[:], Exp, scale=0.125,
                    )
                    pv(g, pr, 0)

                # diagonal key tiles r: causal q-range [128r, 512)
                for r in range(4):
                    o = 128 * r
                    g = 4 * c + r
                    sc_ps = psS.tile([128, 1024], F32, tag="scps", name=f"sd{m}_{c}_{r}")
                    nc.tensor.matmul(
                        sc_ps[:, o:512],
                        lhsT=krot[0:64, g * 128 : (g + 1) * 128],
                        rhs=q0[:, o:512], start=True, stop=True,
                    )
                    nc.tensor.matmul(
                        sc_ps[:, 512 + o : 1024],
                        lhsT=krot[64:128, g * 128 : (g + 1) * 128],
                        rhs=q1[:, o:512], start=True, stop=True,
                    )
                    pr = probsp.tile([128, 1024], BF16, tag="pr", name=f"pd{m}_{c}_{r}")
                    nc.scalar.activation(
                        pr[:, o:1024], sc_ps[:, o:1024], Exp, scale=0.125,
                    )
                    # mask the diagonal 128-block of each sub
                    nc.vector.tensor_tensor(
                        pr[:, o : o + 128], pr[:, o : o + 128], tri_sb[:],
                        op=mybir.AluOpType.mult,
                    )
                    nc.vector.tensor_tensor(
                        pr[:, 512 + o : 512 + o + 128],
                        pr[:, 512 + o : 512 + o + 128], tri_sb[:],
                        op=mybir.AluOpType.mult,
                    )
                    pv(g, pr, o)

                # ---- normalize: attn = pv * (1/den) ----
                # (reciprocal_approx_fast needs a partition-0 f32 SBUF input)
                # high prio: this chain frees the pv PSUM banks and gates
                # the wo matmuls of the chunk
                with tc.high_priority():
                    den = nrm.tile([1, 1024], F32, tag="den", name=f"dn{m}_{c}")
                    nc.any.tensor_copy(den[:], out_ps[64:65, :])
                    pvraw = pvp.tile([65, 1024], BF16, tag="pvraw", name=f"pv{m}_{c}")
                    nc.any.tensor_copy(pvraw[:], out_ps[:])
                    rec_f = nrm.tile([1, 1024], F32, tag="recf", name=f"rf{m}_{c}")
                    nc.vector.reciprocal_approx_fast(rec_f[:], den[:])
                    rec = nrm.tile([1, 1024], BF16, tag="rec", name=f"rc{m}_{c}")
                    nc.vector.tensor_copy(rec[:], rec_f[:])
                    rec64 = nrm.tile([64, 1024], BF16, tag="rec64", name=f"rb{m}_{c}")
                    nc.gpsimd.partition_broadcast(rec64[:], rec[:])
                    nc.vector.tensor_tensor(
                        attn[0:64, m * S + c * 512 : m * S + (c + 1) * 512],
                        pvraw[0:64, 0:512], rec64[:, 0:512],
                        op=mybir.AluOpType.mult,
                    )
                    nc.vector.tensor_tensor(
                        attn[64:128, m * S + c * 512 : m * S + (c + 1) * 512],
                        pvraw[0:64, 512:1024], rec64[:, 512:1024],
                        op=mybir.AluOpType.mult,
                    )

        def wo_chunk(cw):
            for st in range(4 * cw, 4 * cw + 4):
                for half in range(2):
                    ot = p3.tile([128, 1024], BF16, tag="ot", name=f"ot{st}_{half}")
                    for nkh in range(2):
                        nk = half * 2 + nkh
                        ps = psP.tile([128, 512], F32, tag="pj", name=f"wo{st}_{nk}")
                        for kt in range(4):
                            nc.tensor.matmul(
                                ps[:],
                                lhsT=attn[:, kt * S + st * 128 : kt * S + st * 128 + 128],
                                rhs=wo_sb[:, kt * 2048 + nk * 512 : kt * 2048 + (nk + 1) * 512],
                                start=(kt == 0),
                                stop=(kt == 3),
                            )
                        nc.any.tensor_copy(ot[:, nkh * 512 : (nkh + 1) * 512], ps[:])
                    nc.sync.dma_start(
                        out_d[st * 128 : (st + 1) * 128, half * 1024 : (half + 1) * 1024],
                        ot[:],
                    )

        # ---- software-pipelined emission ----
        proj_chunk(0)
        for c in range(4):
            attention_chunk(c)
            if c < 3:
                proj_chunk(c + 1)
            if c >= 1:
                wo_chunk(c - 1)
        wo_chunk(3)


def _get_compiled():
    global _COMPILED
    if _COMPILED is None:
        _COMPILED = _build()
    return _COMPILED


def _bf16(a):
    import ml_dtypes

    return np.asarray(a, np.float32).astype(ml_dtypes.bfloat16)


def _host_tables():
    invf = ROPE_BASE ** (-np.arange(0, DH, 2, dtype=np.float64) / DH)  # [32]
    t = np.arange(S, dtype=np.float64)
    theta = t[None, :] * invf[:, None]  # [32, S]
    c32 = np.cos(theta)
    s32 = np.sin(theta)
    C = np.empty((128, S), np.float32)
    Sg = np.empty((128, S), np.float32)
    for j in range(2):
        C[j * 64 : j * 64 + 32] = c32
        C[j * 64 + 32 : j * 64 + 64] = c32
        Sg[j * 64 : j * 64 + 32] = s32          # +sin for first half
        Sg[j * 64 + 32 : j * 64 + 64] = -s32    # -sin for second half
    tri = np.triu(np.ones((128, 128), np.float32))  # tri[a,b]=1 iff a<=b
    return C, Sg, tri


# device head order within the 512-wide q shard: m-tile m holds local heads
# (m, m+4) so that the q sub-block partition base (64*sub) equals the kv base.
_PERM_Q = np.array(
    [(m + 4 * sub) * DH + d for m in range(4) for sub in range(2) for d in range(DH)],
    dtype=np.int64,
)


def _rearrange_w(w):  # [2048, 768] -> [128, 12288] m-major
    # device slice for (m, k) is w_dev[:, m*2048 + k*128 : +128]
    return np.ascontiguousarray(
        w.reshape(16, 128, 6, 128).transpose(1, 2, 0, 3).reshape(128, 6 * 2048)
    )


def _rearrange_wo(w):  # [512, 2048] -> [128, 8192]
    return np.ascontiguousarray(
        w.reshape(4, 128, 2048).transpose(1, 0, 2).reshape(128, 4 * 2048)
    )


def _make_in_maps(ins):
    x = np.asarray(ins["x"], np.float32)
    wq = np.asarray(ins["wq"], np.float32)
    wk = np.asarray(ins["wk"], np.float32)
    wv = np.asarray(ins["wv"], np.float32)
    wo = np.asarray(ins["wo"], np.float32)

    C, Sg, tri = _host_tables()
    C, Sg, tri = _bf16(C), _bf16(Sg), _bf16(tri)
    xts = [_bf16(np.ascontiguousarray(x[bi].T)) for bi in range(2)]

    in_maps = []
    for c in range(N_CORES):
        bi, g = c // 4, c % 4
        wq_s = wq[:, g * DQ : (g + 1) * DQ][:, _PERM_Q]
        wk_s = wk[:, g * DKV : (g + 1) * DKV]
        wv_s = wv[:, g * DKV : (g + 1) * DKV]
        wall = _bf16(_rearrange_w(
            np.ascontiguousarray(np.concatenate([wq_s, wk_s, wv_s], axis=1))
        ))
        wo_s = _bf16(_rearrange_wo(np.ascontiguousarray(wo[g * DQ : (g + 1) * DQ, :][_PERM_Q])))
        in_maps.append(
            {
                "xt": xts[bi],
                "wall": wall,
                "wo": wo_s,
                "cos": C,
                "sin": Sg,
                "tri": tri,
            }
        )
    return in_maps


def kernel(x, wq, wk, wv, wo):
    global LAST_RESULTS
    nc = _get_compiled()
    in_maps = _make_in_maps({"x": x, "wq": wq, "wk": wk, "wv": wv, "wo": wo})
    res = run_bass_kernel_spmd(nc, in_maps, list(range(N_CORES)), **RUN_KWARGS)
    LAST_RESULTS = res
    out = np.empty((2, S, D), np.float32)
    for bi in range(2):
        acc = np.asarray(res.results[4 * bi]["out"], np.float32)
        for g in range(1, 4):
            acc = acc + np.asarray(res.results[4 * bi + g]["out"], np.float32)
        out[bi] = acc
    return out
